# revision 2
# baseline (speedup 1.0000x reference)
"""TRN2 Bass kernel for nn_ComboFwdVecComp (B=4, S=512, C=V=128).

out[b,i,j,v] = tanh( sum_c ctx[b,i,c] * ( Wm[v,c]*ctx[b,j,c] + (W2-Wd)[v,c] )
                     + A[b,j,v] + btot[v] )
  A = ctx @ (W1+Wd).T  (j-dep affine part),  btot = b1+b2+bm+bd.
The i-dep affine part (ctx_i @ (W2-Wd).T) folds into the main GEMM via the
(W2-Wd).T rhs term; the j-dep part becomes the K=1 bias matmul row.

Output (4,512,512,128) -> memory/ACT bound. This fp16 version stores the
output as fp16 (tanh is bounded, fp16 quantization ~2.4e-4 abs err; host
upcasts to f32), halving HBM store traffic vs f32 (64 -> 32 MiB/core,
~94us DMA floor), and runs the PE in fp16 (1 col/cycle vs f32r's ~2x
slower cadence). The pacer becomes the ACT engine: tanh [128,2048] f32->
fp16 is ~1.8us/half x 64 halves ~ 115-150us.

Sharding: 8 cores, core k handles b = k//2, i in [ (k%2)*256, +256 ).
Each core emits out_shard (256, 512, 128) fp16 = 32 MiB; host concatenates
and upcasts.

i-MAJOR orientation: PSUM partition dim = i, free dim = (j, v) j-major.
For fixed i, 16 consecutive j x 128 v = 4 KiB fp16 is CONTIGUOUS in
out[i,j,v]; HWDGE gets a 3D AP (outer=128 partitions) via
max_dma_last_dim=1024 -> 2 KiB descriptors spread over the 16 SDMA
engines; SWDGE (gpsimd) shreds its plain 2D form itself.

DVE prep runs in fp16 2x_1P mode (2 elem/cycle) by using a (v-outer,
j-inner) rhs' layout: rhs'[c, v*8+j] = Wm[v,c]*ctx[j,c] + W2md[v,c].
In this layout the ctx_j operand's innermost dim is j (step 1, count 8)
with the v-broadcast on the middle (step 0) dim -- 2x_1P only requires
the innermost step to be +-1 with a 2-byte dtype. Wm/W2md are
pre-replicated 8x along j on the host (WMREP/W2MDREP [C, 8*V]) so all
tensor operands are contiguous fp16. The main matmul reads the quad
column block with a strided AP ([1,4],[8,128]) to restore (j,v) v-inner
PSUM column order.

Per-core structure: loop j-groups (jg = 32 j's = 4 pairs of 8), then
i-chunks (ic = 128 i's), then halves (4 banks = 16 j's):
  bias mm (K=1, N=512): ones^T @ browp_quad, strip-tiled on PE rows
      0/32/64/96 so the four bias mms run concurrently.
  main mm (K=128, N=512): ctxiT^T @ rhs'_quad accumulates on top, one
      LDW per half, fp16 at 1 col/cycle.
  ACT tanh drains the half [128,2048] f32 -> fp16 SBUF; ONE 0.5 MiB DMA
  stores it, rotating across 3 queues (SP-HWDGE / Pool-SWDGE / ACT-HWDGE).

A dummy tanh at build start preloads the ACT lookup table (otherwise the
first drain stalls ~9us mid-pipeline). Input DMA order per queue =
modeled completion order (Tile bakes it into semaphore waits).
"""

import sys
import types
from contextlib import ExitStack

import numpy as np

import concourse.bass as bass
import concourse.mybir as mybir
import concourse.tile as tile
from concourse import bacc
from concourse.bass_utils import run_bass_kernel_spmd

B, S, C, V = 4, 512, 128, 128
NCORES = 8
NI = 256          # i's per core
NQJ = S // 4      # j quads (128)
NJG = NQJ // 8    # j groups of 8 quads / 32 j's (16)

_F32 = mybir.dt.float32
_F16 = mybir.dt.float16


def install_ntff_shim():
    """antenv.axon_hooks is absent on some images; shim it so trace=True works."""
    if "antenv.axon_hooks" in sys.modules:
        return
    try:
        from trn_agent_boot.trn_boot import _ntff_profile_via_ctypes
        hook = _ntff_profile_via_ctypes("/opt/axon/libaxon_pjrt.so")
    except Exception:
        hook = None
    mod = types.ModuleType("antenv.axon_hooks")
    mod.get_axon_ntff_profile_hook = lambda: hook
    mod.set_axon_ntff_profile_hook = lambda h: None
    sys.modules["antenv.axon_hooks"] = mod


def build_nc():
    nc = bacc.Bacc("TRN2", target_bir_lowering=False, debug=False)

    BPW = (NQJ // 4) * 512  # browp row width (16384)

    ctxT_d = nc.dram_tensor("ctxT", [C, S], _F16, kind="ExternalInput").ap()
    ctxiT_d = nc.dram_tensor("ctxiT", [C, NI], _F16, kind="ExternalInput").ap()
    wmrep_d = nc.dram_tensor("wmrep", [C, 8 * V], _F16, kind="ExternalInput").ap()
    w2mdrep_d = nc.dram_tensor("w2mdrep", [C, 8 * V], _F16, kind="ExternalInput").ap()
    browp_d = nc.dram_tensor("browp", [4, BPW], _F16, kind="ExternalInput").ap()
    out_d = nc.dram_tensor("out_shard", [NI, S, V], _F16, kind="ExternalOutput").ap()

    with tile.TileContext(nc) as tc, ExitStack() as ctx:
        singles = ctx.enter_context(tc.tile_pool(name="singles", bufs=1))
        rhs_pool = ctx.enter_context(tc.tile_pool(name="rhs", bufs=8))
        tmp_pool = ctx.enter_context(tc.tile_pool(name="tmp", bufs=3))
        psum_pool = ctx.enter_context(tc.tile_pool(name="psum", bufs=1, space="PSUM"))
        out_pool = ctx.enter_context(tc.tile_pool(name="outs", bufs=6))

        # ---- load constants. Queue order = modeled completion order (the
        # Tile scheduler bakes it into semaphore waits): browp rows first
        # (the first bias mms gate on them), then the rhs'-prep deps
        # (wmrep/w2mdrep/ctxT cols 0:32) and ctxiT, then the ctxT bulk. ----
        ctxT_sb = singles.tile([C, S], _F16)
        browp_r = singles.tile([97, BPW], _F16)
        wmrep_sb = singles.tile([C, 8 * V], _F16)
        w2mdrep_sb = singles.tile([C, 8 * V], _F16)
        ctxiT_r = singles.tile([C, NI], _F16)
        for r in range(4):
            eng = nc.sync if r % 2 == 0 else nc.scalar
            eng.dma_start(out=browp_r[32 * r:32 * r + 1, :], in_=browp_d[r:r + 1, :])
        nc.scalar.dma_start(out=wmrep_sb, in_=wmrep_d)
        nc.sync.dma_start(out=w2mdrep_sb, in_=w2mdrep_d)
        nc.scalar.dma_start(out=ctxT_sb[:, 0:32], in_=ctxT_d[:, 0:32])
        nc.sync.dma_start(out=ctxiT_r, in_=ctxiT_d)
        nc.scalar.dma_start(out=ctxT_sb[:, 32:], in_=ctxT_d[:, 32:])

        ones_r = singles.tile([97, 128], _F16)
        nc.vector.memset(ones_r, 1.0)
        # Dummy activation: the ACT engine loads its tanh lookup table on
        # first use (~9us stall observed mid-pipeline); trigger the load now
        # so it overlaps the input DMAs instead of stalling the first drain.
        warm = singles.tile([97, 8], _F32)
        nc.scalar.activation(
            warm, ones_r[:, 0:8], mybir.ActivationFunctionType.Tanh
        )

        # one 8-bank psum megatile; bank b occupies [:, b*512:(b+1)*512]
        P = psum_pool.tile([128, 4096], _F32, name="mega")

        dma_engines = [nc.sync, nc.gpsimd, nc.scalar]
        dma_i = 0

        def prep_pair(gp):
            # rhs' for j's [8*gp, 8*gp+8) in (v-outer, j-inner) layout:
            # rhs[c, v*8+j] = wmrep[c, v*8+j]*ctx[j, c] + w2mdrep[c, v*8+j].
            # ctx_j operand: innermost j (step 1, count 8), v-broadcast on
            # the middle step-0 dim -> fp16 2x_1P on both DVE ops.
            tmp_p = tmp_pool.tile([C, 8 * V], _F16)
            ctxj_bc = bass.AP(
                tensor=ctxT_sb.tensor,
                offset=ctxT_sb.offset + 8 * gp,
                ap=[ctxT_sb.ap[0], [0, V], [1, 8]],
            )
            nc.vector.tensor_tensor(
                out=tmp_p, in0=wmrep_sb, in1=ctxj_bc, op=mybir.AluOpType.mult
            )
            rhs_p = rhs_pool.tile([C, 8 * V], _F16)
            nc.vector.tensor_tensor(
                out=rhs_p, in0=tmp_p, in1=w2mdrep_sb, op=mybir.AluOpType.add
            )
            return rhs_p

        def quad_slice(pairs, qq):
            # quad qq's 512 columns in (j,v) v-inner order out of the
            # (v-outer, j-inner) pair buffer: offset 4*(qq%2), j stride 1
            # (count 4) outer, v stride 8 (count 128) inner.
            rp = pairs[qq // 2]
            return bass.AP(
                tensor=rp.tensor,
                offset=rp.offset + 4 * (qq % 2),
                ap=[rp.ap[0], [1, 4], [8, V]],
            )

        for jg in range(NJG):
            if jg == 0:
                # ramp: only the first half's quads before the first matmuls
                pairs = [prep_pair(0), prep_pair(1), None, None]
            else:
                pairs = [prep_pair(4 * jg + pp) for pp in range(4)]

            for ic in range(2):
                for half in range(2):
                    # ---- bias mms: 4 quads, strip-concurrent ----
                    for s in range(4):
                        q = 8 * jg + 4 * half + s
                        strip = (q % 4) * 32
                        col = (q // 4) * 512
                        bank = 4 * half + s
                        nc.tensor.matmul(
                            P[:, bank * 512:(bank + 1) * 512],
                            lhsT=ones_r[strip:strip + 1, :],
                            rhs=browp_r[strip:strip + 1, col:col + 512],
                            start=True,
                            stop=False,
                            tile_position=(strip, 0),
                        )
                    # ---- main mms: one ctxiT LDW per half ----
                    for s in range(4):
                        bank = 4 * half + s
                        nc.tensor.matmul(
                            P[:, bank * 512:(bank + 1) * 512],
                            lhsT=ctxiT_r[:, ic * 128:(ic + 1) * 128],
                            rhs=quad_slice(pairs, 4 * half + s),
                            start=False,
                            stop=True,
                        )

                    if jg == 0 and ic == 0 and half == 0:
                        pairs[2] = prep_pair(2)
                        pairs[3] = prep_pair(3)

                    # ---- drain the half: tanh [128,2048] f32->fp16 + ONE
                    # 0.5 MiB DMA. HWDGE queues need a 3D AP (outer=128
                    # partitions) to spread descriptors across the 16 SDMA
                    # engines -- a 2D row-list pins the whole chain on one
                    # engine. max_dma_last_dim=1024 -> [[1024,2],[1,1024]]:
                    # 2 KiB descriptors. SWDGE (gpsimd) shreds any shape
                    # itself, so it takes the plain 2D form.
                    ot = out_pool.tile([128, 2048], _F16)
                    nc.scalar.activation(
                        ot, P[:, half * 2048:(half + 1) * 2048],
                        mybir.ActivationFunctionType.Tanh,
                    )
                    j0 = jg * 32 + half * 16
                    dst = bass.AP(
                        tensor=out_d.tensor,
                        offset=(ic * 128) * S * V + j0 * V,
                        ap=[[S * V, 128], [1, 16 * V]],
                    )
                    eng = dma_engines[dma_i % 3]
                    dma_i += 1
                    if eng is nc.gpsimd:
                        eng.dma_start(out=dst, in_=ot[:, :])
                    else:
                        eng.dma_start(out=dst, in_=ot[:, :], max_dma_last_dim=1024)

    nc.compile()
    return nc


_NC_CACHE = {}


def get_nc():
    if "nc" not in _NC_CACHE:
        _NC_CACHE["nc"] = build_nc()
    return _NC_CACHE["nc"]


def make_in_maps(ctx, W1, b1, W2, b2, Wm, bm, Wd, bd):
    ctx = np.asarray(ctx, np.float32)
    btot = (
        np.asarray(b1) + np.asarray(b2) + np.asarray(bm) + np.asarray(bd)
    ).astype(np.float32)
    wmT = np.ascontiguousarray(np.asarray(Wm, np.float32).T)                  # (C,V)
    w2mdT = np.ascontiguousarray(
        (np.asarray(W2) - np.asarray(Wd)).T.astype(np.float32)
    )
    w1d = (np.asarray(W1) + np.asarray(Wd)).astype(np.float32)                # (V,C)

    # replicate 8x along a trailing j dim: rep[c, v*8+j] = src[c, v]
    def rep8(a):
        return np.ascontiguousarray(
            np.repeat(a[:, :, None], 8, axis=2).reshape(C, V * 8).astype(np.float16)
        )

    wmrep = rep8(wmT)
    w2mdrep = rep8(w2mdT)

    per_b = []
    for b in range(B):
        A = (ctx[b] @ w1d.T + btot).astype(np.float32)                        # (S,V)
        browq = A.reshape(NQJ, 4 * V)                                         # quad rows
        browp = np.zeros((4, (NQJ // 4) * 512), np.float16)
        for q in range(NQJ):
            browp[q % 4, (q // 4) * 512:(q // 4) * 512 + 512] = browq[q]
        per_b.append((np.ascontiguousarray(ctx[b].T.astype(np.float16)), browp))

    in_maps = []
    for k in range(NCORES):
        b = k // 2
        i0c = (k % 2) * NI
        ctxT, browp = per_b[b]
        in_maps.append({
            "ctxT": ctxT,
            "ctxiT": np.ascontiguousarray(ctx[b, i0c:i0c + NI].T.astype(np.float16)),
            "wmrep": wmrep,
            "w2mdrep": w2mdrep,
            "browp": browp,
        })
    return in_maps


def run(in_maps, **kw):
    return run_bass_kernel_spmd(get_nc(), in_maps, core_ids=list(range(NCORES)), **kw)


def assemble(results):
    out = np.empty((B, S, S, V), np.float32)
    for k in range(NCORES):
        b = k // 2
        i0c = (k % 2) * NI
        out[b, i0c:i0c + NI] = np.asarray(results[k]["out_shard"], np.float32)
    return out


def kernel(ctx, W1, b1, W2, b2, Wm, bm, Wd, bd):
    install_ntff_shim()
    in_maps = make_in_maps(ctx, W1, b1, W2, b2, Wm, bm, Wd, bd)
    res = run(in_maps)
    return assemble(res.results)


# revision 5
# speedup vs baseline: 1.8625x; 1.8625x over previous
"""TRN2 Bass kernel for nn_ComboFwdVecComp (B=4, S=512, C=V=128).

out[b,i,j,v] = tanh( sum_c ctx[b,i,c] * ( Wm[v,c]*ctx[b,j,c] + (W2-Wd)[v,c] )
                     + A[b,j,v] + btot[v] )
  A = ctx @ (W1+Wd).T  (j-dep affine part),  btot = b1+b2+bm+bd.
The i-dep affine part (ctx_i @ (W2-Wd).T) folds into the main GEMM via the
(W2-Wd).T rhs term; the j-dep part becomes the K=1 bias matmul row.

fp16 version: output stored as fp16 (tanh is bounded; fp16 quantization
adds ~2.4e-4 abs err; host upcasts to f32), halving HBM store traffic vs
f32 (64 -> 32 MiB/core). PE runs fp16 (1 col/cycle warm vs f32r ~2x
slower). Measured rel err ~4e-3 vs the 2e-2 gate.

The pacer is the ACT engine: tanh [128,2048] f32->fp16 at ~1.9us per
half x 64 halves ~ 125us/core. Everything else is kept under that pace:
  - main mm rhs MUST be contiguous (j-major, v-inner pair buffer): a
    strided rhs AP ([1,4],[8,128]) measured 922ns per 512-col fp16 mm
    vs ~213ns contiguous -- the moving-operand SBUF feed collapses on
    16-byte-strided reads.
  - DVE prep: in (j,v) layout the mult's ctx_j operand has a step-0
    innermost dim (v-broadcast) -> 1x mode (~1.3us per [C,1024] op), so
    the mult is SPLIT: DVE does j 0..3, GPSIMD (Pool TT, ~0.42 of 1
    el/cycle/lane) does j 4..7, concurrently. The add (+W2md, all
    operands contiguous fp16, innermost step 1) runs 2x_1P on DVE
    (~0.68us). DVE ~1.5us/half, Pool ~1.0us/half.
  - store DMAs rotate over sync/vector/scalar HWDGE queues (Pool is
    busy with its mult share; HWDGE needs the 3D AP via
    max_dma_last_dim=1024 to spread descriptors over 16 SDMA engines).

Sharding: 8 cores, core k handles b = k//2, i in [ (k%2)*256, +256 ).
Each core emits out_shard (256, 512, 128) fp16 = 32 MiB; host
concatenates and upcasts.

Per-core structure: loop j-groups (32 j's = 4 pairs of 8), then i-chunks
(128 i's), then halves (4 psum banks = 16 j's):
  bias mm (K=1, N=512): ones^T @ browp_quad, strip-tiled on PE rows
      0/32/64/96 so the four bias mms run concurrently.
  main mm (K=128, N=512): ctxiT^T @ rhs'_quad accumulates on top, one
      LDW per half.
  ACT tanh drains the half [128,2048] f32 -> fp16 SBUF; ONE 0.5 MiB DMA.

A dummy tanh at build start preloads the ACT lookup table (otherwise the
first drain stalls ~9us mid-pipeline). Input DMA order per queue =
modeled completion order (Tile bakes it into semaphore waits).
"""

import sys
import types
from contextlib import ExitStack

import numpy as np

import concourse.bass as bass
import concourse.mybir as mybir
import concourse.tile as tile
from concourse import bacc
from concourse.bass_utils import run_bass_kernel_spmd

B, S, C, V = 4, 512, 128, 128
NCORES = 8
NI = 256          # i's per core
NQJ = S // 4      # j quads (128)
NJG = NQJ // 8    # j groups of 8 quads / 32 j's (16)

POOL_SPLIT = True  # GPSIMD takes j 4..7 of the prep mult

_F32 = mybir.dt.float32
_F16 = mybir.dt.float16


def install_ntff_shim():
    """antenv.axon_hooks is absent on some images; shim it so trace=True works."""
    if "antenv.axon_hooks" in sys.modules:
        return
    try:
        from trn_agent_boot.trn_boot import _ntff_profile_via_ctypes
        hook = _ntff_profile_via_ctypes("/opt/axon/libaxon_pjrt.so")
    except Exception:
        hook = None
    mod = types.ModuleType("antenv.axon_hooks")
    mod.get_axon_ntff_profile_hook = lambda: hook
    mod.set_axon_ntff_profile_hook = lambda h: None
    sys.modules["antenv.axon_hooks"] = mod


def build_nc():
    nc = bacc.Bacc("TRN2", target_bir_lowering=False, debug=False)

    BPW = (NQJ // 4) * 512  # browp row width (16384)

    ctxT_d = nc.dram_tensor("ctxT", [C, S], _F16, kind="ExternalInput").ap()
    ctxiT_d = nc.dram_tensor("ctxiT", [C, NI], _F16, kind="ExternalInput").ap()
    wmq_d = nc.dram_tensor("wmq", [C, 4 * V], _F16, kind="ExternalInput").ap()
    w2mdrep_d = nc.dram_tensor("w2mdrep", [C, 8 * V], _F16, kind="ExternalInput").ap()
    browp_d = nc.dram_tensor("browp", [4, BPW], _F16, kind="ExternalInput").ap()
    out_d = nc.dram_tensor("out_shard", [NI, S, V], _F16, kind="ExternalOutput").ap()

    with tile.TileContext(nc) as tc, ExitStack() as ctx:
        singles = ctx.enter_context(tc.tile_pool(name="singles", bufs=1))
        rhs_pool = ctx.enter_context(tc.tile_pool(name="rhs", bufs=8))
        tmp_pool = ctx.enter_context(tc.tile_pool(name="tmp", bufs=3))
        psum_pool = ctx.enter_context(tc.tile_pool(name="psum", bufs=1, space="PSUM"))
        out_pool = ctx.enter_context(tc.tile_pool(name="outs", bufs=6))

        # ---- load constants. Queue order = modeled completion order (the
        # Tile scheduler bakes it into semaphore waits): browp rows first
        # (the first bias mms gate on them), then the rhs'-prep deps
        # (wmq/w2mdrep/ctxT cols 0:32) and ctxiT, then the ctxT bulk. ----
        ctxT_sb = singles.tile([C, S], _F16)
        browp_r = singles.tile([97, BPW], _F16)
        wmq_sb = singles.tile([C, 4 * V], _F16)
        w2mdrep_sb = singles.tile([C, 8 * V], _F16)
        ctxiT_r = singles.tile([C, NI], _F16)
        for r in range(4):
            eng = nc.sync if r % 2 == 0 else nc.scalar
            eng.dma_start(out=browp_r[32 * r:32 * r + 1, :], in_=browp_d[r:r + 1, :])
        nc.scalar.dma_start(out=wmq_sb, in_=wmq_d)
        nc.sync.dma_start(out=w2mdrep_sb, in_=w2mdrep_d)
        nc.scalar.dma_start(out=ctxT_sb[:, 0:32], in_=ctxT_d[:, 0:32])
        nc.sync.dma_start(out=ctxiT_r, in_=ctxiT_d)
        nc.scalar.dma_start(out=ctxT_sb[:, 32:], in_=ctxT_d[:, 32:])

        ones_r = singles.tile([97, 128], _F16)
        nc.vector.memset(ones_r, 1.0)
        # Dummy activation: the ACT engine loads its tanh lookup table on
        # first use (~9us stall observed mid-pipeline); trigger the load now
        # so it overlaps the input DMAs instead of stalling the first drain.
        warm = singles.tile([97, 8], _F32)
        nc.scalar.activation(
            warm, ones_r[:, 0:8], mybir.ActivationFunctionType.Tanh
        )

        # one 8-bank psum megatile; bank b occupies [:, b*512:(b+1)*512]
        P = psum_pool.tile([128, 4096], _F32, name="mega")

        # only SP and ACT have HWDGE queues (vector can't DMA; Pool/SWDGE is
        # kept free for its share of the prep mult)
        dma_engines = [nc.sync, nc.scalar]
        dma_i = 0

        def ctxj_bc(gp, j0, nj):
            # ctx_j broadcast: j outer (step 1, count nj), v inner (step 0)
            return bass.AP(
                tensor=ctxT_sb.tensor,
                offset=ctxT_sb.offset + 8 * gp + j0,
                ap=[ctxT_sb.ap[0], [1, nj], [0, V]],
            )

        def prep_pair(gp):
            # rhs' for j's [8*gp, 8*gp+8) in (j-major, v-inner) layout:
            # rhs[c, jl*V+v] = wm[c,v]*ctx[8gp+jl, c] + w2md[c,v].
            # The mult is 1x on DVE (step-0 innermost operand), so split it
            # j-wise with GPSIMD; the add is all-contiguous fp16 -> DVE 2x.
            tmp_p = tmp_pool.tile([C, 8 * V], _F16)
            nc.vector.tensor_tensor(
                out=tmp_p[:, 0:4 * V], in0=wmq_sb, in1=ctxj_bc(gp, 0, 4),
                op=mybir.AluOpType.mult,
            )
            if POOL_SPLIT:
                nc.gpsimd.tensor_tensor(
                    out=tmp_p[:, 4 * V:8 * V], in0=wmq_sb, in1=ctxj_bc(gp, 4, 4),
                    op=mybir.AluOpType.mult,
                )
            else:
                nc.vector.tensor_tensor(
                    out=tmp_p[:, 4 * V:8 * V], in0=wmq_sb, in1=ctxj_bc(gp, 4, 4),
                    op=mybir.AluOpType.mult,
                )
            rhs_p = rhs_pool.tile([C, 8 * V], _F16)
            nc.vector.tensor_tensor(
                out=rhs_p, in0=tmp_p, in1=w2mdrep_sb, op=mybir.AluOpType.add
            )
            return rhs_p

        def pair_slice(pairs, qq):
            return pairs[qq // 2][:, (qq % 2) * 4 * V:(qq % 2 + 1) * 4 * V]

        for jg in range(NJG):
            if jg == 0:
                # ramp: only the first half's quads before the first matmuls
                pairs = [prep_pair(0), prep_pair(1), None, None]
            else:
                pairs = [prep_pair(4 * jg + pp) for pp in range(4)]

            for ic in range(2):
                for half in range(2):
                    # ---- bias mms: 4 quads, strip-concurrent ----
                    for s in range(4):
                        q = 8 * jg + 4 * half + s
                        strip = (q % 4) * 32
                        col = (q // 4) * 512
                        bank = 4 * half + s
                        nc.tensor.matmul(
                            P[:, bank * 512:(bank + 1) * 512],
                            lhsT=ones_r[strip:strip + 1, :],
                            rhs=browp_r[strip:strip + 1, col:col + 512],
                            start=True,
                            stop=False,
                            tile_position=(strip, 0),
                        )
                    # ---- main mms: one ctxiT LDW per half ----
                    for s in range(4):
                        bank = 4 * half + s
                        nc.tensor.matmul(
                            P[:, bank * 512:(bank + 1) * 512],
                            lhsT=ctxiT_r[:, ic * 128:(ic + 1) * 128],
                            rhs=pair_slice(pairs, 4 * half + s),
                            start=False,
                            stop=True,
                        )

                    if jg == 0 and ic == 0 and half == 0:
                        pairs[2] = prep_pair(2)
                        pairs[3] = prep_pair(3)

                    # ---- drain the half: tanh [128,2048] f32->fp16 + ONE
                    # 0.5 MiB DMA. HWDGE queues need a 3D AP (outer=128
                    # partitions) to spread descriptors across the 16 SDMA
                    # engines -- a 2D row-list pins the whole chain on one
                    # engine. max_dma_last_dim=1024 -> [[1024,2],[1,1024]]:
                    # 2 KiB descriptors.
                    ot = out_pool.tile([128, 2048], _F16)
                    nc.scalar.activation(
                        ot, P[:, half * 2048:(half + 1) * 2048],
                        mybir.ActivationFunctionType.Tanh,
                    )
                    j0 = jg * 32 + half * 16
                    dst = bass.AP(
                        tensor=out_d.tensor,
                        offset=(ic * 128) * S * V + j0 * V,
                        ap=[[S * V, 128], [1, 16 * V]],
                    )
                    eng = dma_engines[dma_i % 2]
                    dma_i += 1
                    eng.dma_start(out=dst, in_=ot[:, :], max_dma_last_dim=1024)

    nc.compile()
    return nc


_NC_CACHE = {}


def get_nc():
    if "nc" not in _NC_CACHE:
        _NC_CACHE["nc"] = build_nc()
    return _NC_CACHE["nc"]


def make_in_maps(ctx, W1, b1, W2, b2, Wm, bm, Wd, bd):
    ctx = np.asarray(ctx, np.float32)
    btot = (
        np.asarray(b1) + np.asarray(b2) + np.asarray(bm) + np.asarray(bd)
    ).astype(np.float32)
    wmT = np.ascontiguousarray(np.asarray(Wm, np.float32).T)                  # (C,V)
    w2mdT = np.ascontiguousarray(
        (np.asarray(W2) - np.asarray(Wd)).T.astype(np.float32)
    )
    w1d = (np.asarray(W1) + np.asarray(Wd)).astype(np.float32)                # (V,C)

    wmq = np.ascontiguousarray(np.tile(wmT, (1, 4)).astype(np.float16))       # (C,4V)
    w2mdrep = np.ascontiguousarray(np.tile(w2mdT, (1, 8)).astype(np.float16))  # (C,8V)

    per_b = []
    for b in range(B):
        A = (ctx[b] @ w1d.T + btot).astype(np.float32)                        # (S,V)
        browq = A.reshape(NQJ, 4 * V)                                         # quad rows
        browp = np.zeros((4, (NQJ // 4) * 512), np.float16)
        for q in range(NQJ):
            browp[q % 4, (q // 4) * 512:(q // 4) * 512 + 512] = browq[q]
        per_b.append((np.ascontiguousarray(ctx[b].T.astype(np.float16)), browp))

    in_maps = []
    for k in range(NCORES):
        b = k // 2
        i0c = (k % 2) * NI
        ctxT, browp = per_b[b]
        in_maps.append({
            "ctxT": ctxT,
            "ctxiT": np.ascontiguousarray(ctx[b, i0c:i0c + NI].T.astype(np.float16)),
            "wmq": wmq,
            "w2mdrep": w2mdrep,
            "browp": browp,
        })
    return in_maps


def run(in_maps, **kw):
    return run_bass_kernel_spmd(get_nc(), in_maps, core_ids=list(range(NCORES)), **kw)


def assemble(results):
    out = np.empty((B, S, S, V), np.float32)
    for k in range(NCORES):
        b = k // 2
        i0c = (k % 2) * NI
        out[b, i0c:i0c + NI] = np.asarray(results[k]["out_shard"], np.float32)
    return out


def kernel(ctx, W1, b1, W2, b2, Wm, bm, Wd, bd):
    install_ntff_shim()
    in_maps = make_in_maps(ctx, W1, b1, W2, b2, Wm, bm, Wd, bd)
    res = run(in_maps)
    return assemble(res.results)
